# revision 39
# baseline (speedup 1.0000x reference)
"""Trainium2 Bass kernel for nn_MultiHeadAttention_32031866093611.

Sharding: pure data parallel — batch b -> NeuronCore b (B == n_cores == 8).
Weights replicated. No collectives.

Per-core program (batch b, S=1024, D=1024, H=16, DK=64):

  qT[c]   = (Wq[:, c*128:+128]).T @ xT + bq  -> [128 d', 1024 s] bf16  (fp32r mm)
  kT[c]   = same with Wk                     -> [128 d', 1024 s] bf16
  v[sc]   = (xT[:, sc*128:+128]).T @ Wv + bv -> [128 s, 16, 128] bf16
            (cols 64:128 memset to 1.0 so PV emits the softmax denominator
             replicated across 64 partitions for free — no partition bcast)
  per head h (c=h//2, r=h%2*64), kc DESCENDING 7..0:
      sT[kc] = kT[c][r:r+64, kc*128:+128].T @ qT[c][r:r+64, 0:hi]  # [128 k, hi q]
      eT[kc] = exp(sT[kc])  (ACT, psum->sbuf, bf16)
      eT[kc] *= 0/1 bf16 mask on cols [kc*128, hi)  (DVE 2x bf16 mode)
      outT  += v[kc][:, h, :].T @ eT[kc][:, 0:hi]  # [128, 1024]: 64:128 = denom
    attnT[c][r:r+64, :] = outT[0:64, :] * (1/outT[64:128, :])  -> bf16
  out[sc] = (attnT[.][:, sc*128:+128]).T @ Wo16 + bo  (bias + DMA per half)

hi = hi_kc = max(TRIM, (kc+1)*128) clipped to 1024, TRIM = max(prefix): cols
q >= TRIM strictly below the diagonal are masked for EVERY core, so their
scores/exp/mask/PV work is skipped; descending kc makes the per-tile PV
column ranges nest, keeping psum accumulation start/stop valid. The program
is rebuilt (cached) per distinct TRIM.

Schedule: flat (h, kc) stream with PV lagging scores/exp; o_proj chunk k
(heads 2k, 2k+1) emitted two heads later, chunk 6 held back to cover the
final norm window; warm-up matmuls on a memset tile keep the PE clock ramp
hot from t~0; startup DMAs ordered so the first projection matmul's operands
(x chunk 0 + first Wq strip) land first.
"""

from collections import deque

import numpy as np
import ml_dtypes

import concourse.bass as bass
import concourse.mybir as mybir
import concourse.tile as tile
from concourse import bacc
from concourse.bass_utils import run_bass_kernel_spmd

B, S, D, H = 8, 1024, 1024, 16
DK = D // H  # 64
P = 128
NCHUNK = S // P  # 8
NCORES = 8
F32R = mybir.dt.float32r
F32 = mybir.dt.float32
BF16 = mybir.dt.bfloat16
EXP = mybir.ActivationFunctionType.Exp
HALF = 512  # fp32 moving-operand / psum-bank max

_CACHED = {}


def _tile_hi(trim):
    """Per-kc live column bound: cols q >= max(trim, (kc+1)*128) are dead."""
    return [min(max(trim, (kc + 1) * P), S) for kc in range(NCHUNK)]


def build_nc(trim=S, repeats=1, parts=frozenset({"scores", "exp", "mask", "pv"})):
    hi = _tile_hi(trim)
    hi_s = hi if "scores" in parts else [S] * NCHUNK
    hi_e = hi if "exp" in parts else [S] * NCHUNK
    hi_p = hi if "pv" in parts else [S] * NCHUNK
    assert all(p <= e <= s for p, e, s in zip(hi_p, hi_e, hi_s))
    msk_off = {}
    off = 0
    for kc in range(NCHUNK):
        msk_off[kc] = off
        off += hi[kc] - kc * P
    msk_len = off

    nc = bacc.Bacc("TRN2", target_bir_lowering=False, debug=False, num_devices=NCORES)

    xt_d = nc.dram_tensor("xt", [D, S], F32R, kind="ExternalInput").ap()
    wq_d = nc.dram_tensor("wq", [D, D], F32R, kind="ExternalInput").ap()
    wk_d = nc.dram_tensor("wk", [D, D], F32R, kind="ExternalInput").ap()
    wv_d = nc.dram_tensor("wv", [D, D], F32R, kind="ExternalInput").ap()
    wo_d = nc.dram_tensor("wo", [D, D], BF16, kind="ExternalInput").ap()
    bqk_d = nc.dram_tensor("bqk", [P, 2 * NCHUNK], F32, kind="ExternalInput").ap()
    bv_d = nc.dram_tensor("bv", [P, D], F32, kind="ExternalInput").ap()
    bo_d = nc.dram_tensor("bo", [P, D], F32, kind="ExternalInput").ap()
    msk_d = nc.dram_tensor("mask8", [P, msk_len], BF16, kind="ExternalInput").ap()
    out_d = nc.dram_tensor("out", [S, D], F32, kind="ExternalOutput").ap()

    with tile.TileContext(nc) as tc:
        with (
            tc.tile_pool(name="w", bufs=18) as wpool,
            tc.tile_pool(name="big", bufs=2) as bigpool,
            tc.tile_pool(name="qk", bufs=8) as qkpool,
            tc.tile_pool(name="v", bufs=8) as vpool,
            tc.tile_pool(name="cst", bufs=1) as cstpool,
            tc.tile_pool(name="exp", bufs=5) as exppool,
            tc.tile_pool(name="rcp", bufs=1) as rcppool,
            tc.tile_pool(name="osb", bufs=3) as osbpool,
            tc.tile_pool(name="pp", bufs=2, space="PSUM") as pp,
            tc.tile_pool(name="po", bufs=2, space="PSUM") as po,
        ):
            for _rep in range(repeats):
                # ---- warm-up on a memset tile: PE ramping from t~0 ----
                wup = cstpool.tile([P, P], BF16, tag="wup")
                nc.vector.memset(wup[:], 0.0)
                wps = pp.tile([P, S], F32, tag="pp", name="warmup_ps")
                for wi in range(34):
                    nc.tensor.matmul(
                        wps[:, 0:P], wup[:], wup[:], start=True, stop=True
                    )

                # ---- startup DMAs, ordered by first use ----
                xtq = [
                    bigpool.tile([P, 4, S], F32R, tag="big", name=f"xtq_{g}")
                    for g in range(2)
                ]

                def x_part(g, hf, c0, c1):
                    """One DMA for x chunks 4g+c0..4g+c1-1, column half hf."""
                    sl = slice(hf * HALF, (hf + 1) * HALF)
                    nc.sync.dma_start(
                        xtq[g][:, c0:c1, sl],
                        xt_d[
                            g * HALF + c0 * P : g * HALF + c1 * P, sl
                        ].rearrange("(c p) q -> p c q", p=P),
                    )

                def whalf(nm, w_dram, hf, lo=0, hi_=NCHUNK, dt=F32R):
                    """Half-strips [128, 512] of W columns [hf*512, (hf+1)*512)."""
                    ts = []
                    sl = slice(hf * HALF, (hf + 1) * HALF)
                    for dc in range(lo, hi_):
                        t = wpool.tile([P, HALF], dt, tag="w", name=f"{nm}{hf}_{dc}")
                        nc.sync.dma_start(t[:], w_dram[dc * P : (dc + 1) * P, sl])
                        ts.append(t)
                    return ts

                x_part(0, 0, 0, 1)
                qh0 = whalf("wq", wq_d, 0, 0, 1)
                x_part(0, 0, 1, 4)
                qh0 += whalf("wq", wq_d, 0, 1, 4)
                x_part(1, 0, 0, 4)
                qh0 += whalf("wq", wq_d, 0, 4, 8)
                bqk = cstpool.tile([P, 2 * NCHUNK], F32, tag="bqk")
                nc.sync.dma_start(bqk[:], bqk_d[:])
                x_part(0, 1, 0, 4)
                x_part(1, 1, 0, 4)
                kh0 = whalf("wk", wk_d, 0)
                bias = {}
                # bv (v-proj) and bo (o_proj) lifetimes don't overlap: share slot
                bias["bv"] = cstpool.tile([P, D], F32, tag="bvbo", name="bv_bc")
                nc.sync.dma_start(bias["bv"][:], bv_d[:])
                msk = cstpool.tile([P, msk_len], BF16, tag="msk")
                nc.sync.dma_start(msk[:], msk_d[:])
                xt = [xtq[dc // 4][:, dc % 4, :] for dc in range(NCHUNK)]

                # ---- helper: dense [d', s] projection (qT / kT), bf16 out ----
                def proj_half(whalf_tiles, chalf, bcol0, out_tag):
                    """qT/kT chunks chalf*4 .. chalf*4+3 from one W column half."""
                    outs = []
                    for cp in range(2):
                        cs = (chalf * 4 + 2 * cp, chalf * 4 + 2 * cp + 1)
                        pss = {
                            c: pp.tile([P, S], F32, tag="pp", name=f"ps_{out_tag}_{c}")
                            for c in cs
                        }
                        for j in range(2):
                            sl = slice(j * HALF, (j + 1) * HALF)
                            for c in cs:
                                lc = (c % 4) * P
                                for dc in range(NCHUNK):
                                    nc.tensor.matmul(
                                        pss[c][:, sl],
                                        whalf_tiles[dc][:, lc : lc + P],
                                        xt[dc][:, sl],
                                        start=(dc == 0),
                                        stop=(dc == NCHUNK - 1),
                                    )
                        for c in cs:
                            o = qkpool.tile(
                                [P, S], F32R, tag=out_tag, name=f"{out_tag}_{c}"
                            )
                            nc.vector.tensor_add(
                                o[:],
                                pss[c][:],
                                bqk[:, bcol0 + c : bcol0 + c + 1].to_broadcast((P, S)),
                            )
                            outs.append(o)
                    return outs

                with nc.named_scope("qk_proj"):
                    qT = proj_half(qh0, 0, 0, "qT")
                    qh1 = whalf("wq", wq_d, 1)
                    kT = proj_half(kh0, 0, NCHUNK, "kT")
                    kh1 = whalf("wk", wk_d, 1)
                    qT += proj_half(qh1, 1, 0, "qT")
                    kT += proj_half(kh1, 1, NCHUNK, "kT")

                # ---- v projection: [s, 16, 128] bf16, cols 64:128 = 1.0 ----
                with nc.named_scope("v_proj"):
                    vh = [whalf("wv", wv_d, 0), whalf("wv", wv_d, 1)]
                    vtiles = []
                    for sc in range(NCHUNK):
                        vt = vpool.tile([P, H, 2 * DK], BF16, tag="v")
                        nc.vector.memset(vt[:, :, DK : 2 * DK], 1.0)
                        ps = pp.tile([P, S], F32, tag="pp")
                        for j in range(2):
                            sl = slice(j * HALF, (j + 1) * HALF)
                            for dc in range(NCHUNK):
                                nc.tensor.matmul(
                                    ps[:, sl],
                                    xt[dc][:, sc * P : (sc + 1) * P],
                                    vh[j][dc][:],
                                    start=(dc == 0),
                                    stop=(dc == NCHUNK - 1),
                                )
                        nc.vector.tensor_add(
                            vt[:, :, 0:DK],
                            ps[:].rearrange("p (h d) -> p h d", h=H),
                            bias["bv"][:].rearrange("p (h d) -> p h d", h=H),
                        )
                        vtiles.append(vt)

                # ---- attention heads ----
                bias["bo"] = cstpool.tile([P, D], F32, tag="bvbo", name="bo_bc")
                nc.sync.dma_start(bias["bo"][:], bo_d[:])
                attn = [None, None]

                # Wo bf16 strips prefetched before the head loop.
                oh = [whalf("wo", wo_d, 0, dt=BF16), whalf("wo", wo_d, 1, dt=BF16)]

                def emit_scores_exp(h, kc):
                    """scores on PE, exp on ACT (bf16 out), 0/1 bf16 masks on DVE."""
                    c, r = h // 2, (h % 2) * DK
                    pss = pp.tile([P, S], F32, tag="pp", name=f"pss_{h}_{kc}")
                    lhs = kT[c][r : r + DK, kc * P : (kc + 1) * P]
                    for lo in range(0, hi_s[kc], HALF):
                        sl = slice(lo, min(lo + HALF, hi_s[kc]))
                        nc.tensor.matmul(
                            pss[:, sl],
                            lhs,
                            qT[c][r : r + DK, sl],
                            start=True,
                            stop=True,
                        )
                    et = exppool.tile([P, S], BF16, tag="exp", name=f"et_{h}_{kc}")
                    nc.scalar.activation(
                        et[:, 0 : hi_e[kc]], pss[:, 0 : hi_e[kc]], EXP
                    )
                    # one 0/1 mask mult over cols [kc*128, hi): diag pattern on
                    # the diagonal block, column mask below the diagonal
                    w = hi[kc] - kc * P
                    off = msk_off[kc]
                    nc.vector.tensor_mul(
                        et[:, kc * P : hi[kc]],
                        et[:, kc * P : hi[kc]],
                        msk[:, off : off + w],
                    )
                    return et

                def emit_pv(h, kc, pso, et):
                    # kc descending: ranges nest. PSUM start=True zeroing is
                    # BANK-granular, so the first writer (kc=7, whose range is
                    # always full) must cover each 512-col bank in ONE
                    # start=True matmul; later kc's accumulate prefixes of the
                    # bank with start=False. stop=True goes on each bank's
                    # last writer (the smallest kc that still reaches it).
                    first = kc == NCHUNK - 1
                    for b0 in range(0, hi_p[kc], HALF):
                        hi2 = min(b0 + HALF, S) if first else min(b0 + HALF, hi_p[kc])
                        last = all(hi_p[k2] <= b0 for k2 in range(kc))
                        nc.tensor.matmul(
                            pso[:, b0:hi2],
                            vtiles[kc][:, h, :],
                            et[:, b0:hi2],
                            start=first,
                            stop=last,
                        )

                normq = deque()

                def emit_norm(h, pso):
                    """Reciprocal now; queue the 4 scramble muls for staggered
                    emission (drained one per pop) so the DVE isn't bursty at
                    head boundaries, starving the mask-mults that gate PV."""
                    rcp = rcppool.tile([DK, S], F32, tag="rcp", name=f"rcp_{h}")
                    nc.vector.reciprocal(rcp[:], pso[DK : 2 * DK, :])
                    # attn[g][e*64+d, cc, h*64+u] = O_h[u*16 + 2*(4g+cc) + e, d]/denom
                    src = pso[0:DK, :].rearrange("d (u j) -> d j u", j=16)
                    rbs = rcp[:].rearrange("d (u j) -> d j u", j=16)
                    for g in range(2):
                        if attn[g] is None:
                            attn[g] = bigpool.tile(
                                [P, 4, S], BF16, tag="big", name=f"attnq_{g}"
                            )
                        for e in range(2):
                            jsl = slice(8 * g + e, 8 * (g + 1), 2)

                            def mul(g=g, e=e, jsl=jsl, src=src, rbs=rbs, h=h):
                                nc.vector.tensor_mul(
                                    attn[g][
                                        e * DK : (e + 1) * DK, :, h * DK : (h + 1) * DK
                                    ],
                                    src[:, jsl, :],
                                    rbs[:, jsl, :],
                                )

                            normq.append((h, mul))

                def drain_one_norm():
                    if normq:
                        normq.popleft()[1]()

                def flush_norm(upto_h):
                    while normq and normq[0][0] <= upto_h:
                        normq.popleft()[1]()

                oq = deque()

                def emit_oproj(sc, spread=False):
                    """o_proj chunk sc. spread=True queues per-cc pieces that
                    are drained ahead of each tile's scores matmuls, so the PE
                    has work in program order while it waits on the scores
                    psum ring (exp + semaphore round-trip)."""
                    ps = po.tile([P, S], F32, tag="po", name=f"psf_{sc}")
                    for j in range(2):
                        sl = slice(j * HALF, (j + 1) * HALF)
                        for cc in range(NCHUNK):

                            def mm(j=j, cc=cc, sl=sl):
                                nc.tensor.matmul(
                                    ps[:, sl],
                                    attn[cc // 4][:, cc % 4, sc * P : (sc + 1) * P],
                                    oh[j][cc][:],
                                    start=(cc == 0),
                                    stop=(cc == NCHUNK - 1),
                                )

                            oq.append(mm) if spread else mm()

                        def biasdma(j=j, sl=sl):
                            ot = osbpool.tile(
                                [P, HALF], F32, tag="osb", name=f"ot_{sc}_{j}"
                            )
                            nc.vector.tensor_add(ot[:], ps[:, sl], bias["bo"][:, sl])
                            nc.sync.dma_start(out_d[sc * P : (sc + 1) * P, sl], ot[:])

                        oq.append(biasdma) if spread else biasdma()

                def drain_oq(n=2):
                    for _ in range(n):
                        if oq:
                            oq.popleft()()

                # Flat (h, kc-descending) stream, PV lagging scores/exp so the
                # in-order PE never waits on a just-issued exp. o_proj chunk k
                # (needs heads 2k,2k+1 only) runs two heads later; chunk 6 is
                # held for the final norm window so the PE stays busy at the
                # tail (holding more would recycle pso_15's psum slot and
                # serialize on the last norm).
                pend = deque()
                pso_cur = None

                def pop_pv():
                    ph, pkc, ppso, pet = pend.popleft()
                    emit_pv(ph, pkc, ppso, pet)
                    drain_one_norm()
                    if pkc == 0:
                        emit_norm(ph, ppso)
                        if ph % 2 == 1 and 3 <= ph <= 13:
                            sc = (ph - 3) // 2
                            flush_norm(2 * sc + 1)
                            emit_oproj(sc, spread=True)

                for h in range(H):
                    pso_cur = po.tile([P, S], F32, tag="po", name=f"pso_{h}")
                    for kc in range(NCHUNK - 1, -1, -1):
                        drain_oq(1)
                        et = emit_scores_exp(h, kc)
                        if len(pend) >= 4:
                            pop_pv()
                        pend.append((h, kc, pso_cur, et))
                while len(pend) > 1:
                    pop_pv()
                # last PV of head 15: norm emitted first so DVE starts at once;
                # o_proj chunk 6 keeps the PE busy under the final norm chain.
                ph, pkc, ppso, pet = pend.popleft()
                emit_pv(ph, pkc, ppso, pet)
                emit_norm(ph, ppso)
                drain_oq(len(oq))
                flush_norm(H - 2)
                emit_oproj(NCHUNK - 2)
                flush_norm(H)
                emit_oproj(NCHUNK - 1)

    nc.compile()
    return nc


def _host_masks(prefix_b: int, trim: int):
    """Combined multiplicative 0/1 mask, bf16, applied to exp output.

    For scores-T tile kc (cols q in [kc*128, hi_kc)): element (i, q) keeps
    exp iff allowed(q, k=kc*128+i) = (q < prefix) or (k >= q).
    """
    hi = _tile_hi(trim)
    i = np.arange(P)[:, None]
    segs = []
    for kc in range(NCHUNK):
        q = np.arange(kc * P, hi[kc])[None, :]
        k = kc * P + i
        allowed = (q < prefix_b) | (k >= q)
        segs.append(allowed.astype(ml_dtypes.bfloat16))
    return np.concatenate(segs, axis=1)


def kernel(x, prefix, Wq, bq, Wk, bk, Wv, bv, Wo, bo, _trace=False):
    x = np.asarray(x, dtype=np.float32)
    prefix = np.asarray(prefix)
    Wq, Wk, Wv = (
        np.ascontiguousarray(np.asarray(w, np.float32)) for w in (Wq, Wk, Wv)
    )
    Wo16 = np.ascontiguousarray(
        np.asarray(Wo, np.float32).astype(ml_dtypes.bfloat16)
    )
    bv, bo = (
        np.broadcast_to(np.asarray(v, np.float32).reshape(1, D), (P, D)).copy()
        for v in (bv, bo)
    )
    bqk = np.stack(
        [np.asarray(bq, np.float32).reshape(NCHUNK, P), np.asarray(bk, np.float32).reshape(NCHUNK, P)], axis=0
    ).reshape(2 * NCHUNK, P).T.copy()  # [128, 16]: cols 0-7 = bq chunks, 8-15 = bk

    # cols q >= max(prefix) below the diagonal are masked on every core:
    # specialize (and cache) the program on that bound.
    trim = int(prefix.max())
    if _CACHED.get("trim") != trim:
        _CACHED["nc"] = build_nc(trim=trim)
        _CACHED["trim"] = trim
    nc = _CACHED["nc"]

    in_maps = []
    for b in range(B):
        mask8 = _host_masks(int(prefix[b]), trim)
        in_maps.append(
            {
                "xt": np.ascontiguousarray(x[b].T),
                "wq": Wq, "wk": Wk, "wv": Wv, "wo": Wo16,
                "bqk": bqk, "bv": bv, "bo": bo,
                "mask8": mask8,
            }
        )

    res = run_bass_kernel_spmd(nc, in_maps, core_ids=list(range(NCORES)), trace=_trace)
    out = np.stack([res.results[b]["out"] for b in range(B)], axis=0)
    if _trace:
        return out, res
    return out


# revision 40
# speedup vs baseline: 1.0040x; 1.0040x over previous
"""Trainium2 Bass kernel for nn_MultiHeadAttention_32031866093611.

Sharding: pure data parallel — batch b -> NeuronCore b (B == n_cores == 8).
Weights replicated. No collectives.

Per-core program (batch b, S=1024, D=1024, H=16, DK=64):

  qT[c]   = (Wq[:, c*128:+128]).T @ xT + bq  -> [128 d', 1024 s] bf16  (fp32r mm)
  kT[c]   = same with Wk                     -> [128 d', 1024 s] bf16
  v[sc]   = (xT[:, sc*128:+128]).T @ Wv + bv -> [128 s, 16, 128] bf16
            (cols 64:128 memset to 1.0 so PV emits the softmax denominator
             replicated across 64 partitions for free — no partition bcast)
  per head h (c=h//2, r=h%2*64), kc DESCENDING 7..0:
      sT[kc] = kT[c][r:r+64, kc*128:+128].T @ qT[c][r:r+64, 0:hi]  # [128 k, hi q]
      eT[kc] = exp(sT[kc])  (ACT, psum->sbuf, bf16)
      eT[kc] *= 0/1 bf16 mask on cols [kc*128, hi)  (DVE 2x bf16 mode)
      outT  += v[kc][:, h, :].T @ eT[kc][:, 0:hi]  # [128, 1024]: 64:128 = denom
    attnT[c][r:r+64, :] = outT[0:64, :] * (1/outT[64:128, :])  -> bf16
  out[sc] = (attnT[.][:, sc*128:+128]).T @ Wo16 + bo  (bias + DMA per half)

hi = hi_kc = max(TRIM, (kc+1)*128) clipped to 1024, TRIM = max(prefix): cols
q >= TRIM strictly below the diagonal are masked for EVERY core, so their
scores/exp/mask/PV work is skipped; descending kc makes the per-tile PV
column ranges nest, keeping psum accumulation start/stop valid. The program
is rebuilt (cached) per distinct TRIM.

Schedule: flat (h, kc) stream with PV lagging scores/exp; o_proj chunk k
(heads 2k, 2k+1) emitted two heads later, chunk 6 held back to cover the
final norm window; warm-up matmuls on a memset tile keep the PE clock ramp
hot from t~0; startup DMAs ordered so the first projection matmul's operands
(x chunk 0 + first Wq strip) land first.
"""

from collections import deque

import numpy as np
import ml_dtypes

import concourse.bass as bass
import concourse.mybir as mybir
import concourse.tile as tile
from concourse import bacc
from concourse.bass_utils import run_bass_kernel_spmd

B, S, D, H = 8, 1024, 1024, 16
DK = D // H  # 64
P = 128
NCHUNK = S // P  # 8
NCORES = 8
F32R = mybir.dt.float32r
F32 = mybir.dt.float32
BF16 = mybir.dt.bfloat16
EXP = mybir.ActivationFunctionType.Exp
HALF = 512  # fp32 moving-operand / psum-bank max

_CACHED = {}


def _tile_hi(trim):
    """Per-kc live column bound: cols q >= max(trim, (kc+1)*128) are dead."""
    return [min(max(trim, (kc + 1) * P), S) for kc in range(NCHUNK)]


def build_nc(trim=S, repeats=1, parts=frozenset({"scores", "exp", "mask", "pv"})):
    hi = _tile_hi(trim)
    hi_s = hi if "scores" in parts else [S] * NCHUNK
    hi_e = hi if "exp" in parts else [S] * NCHUNK
    hi_p = hi if "pv" in parts else [S] * NCHUNK
    assert all(p <= e <= s for p, e, s in zip(hi_p, hi_e, hi_s))
    msk_off = {}
    off = 0
    for kc in range(NCHUNK):
        msk_off[kc] = off
        off += hi[kc] - kc * P
    msk_len = off

    nc = bacc.Bacc("TRN2", target_bir_lowering=False, debug=False, num_devices=NCORES)

    xt_d = nc.dram_tensor("xt", [D, S], F32R, kind="ExternalInput").ap()
    wq_d = nc.dram_tensor("wq", [D, D], F32R, kind="ExternalInput").ap()
    wk_d = nc.dram_tensor("wk", [D, D], F32R, kind="ExternalInput").ap()
    wv_d = nc.dram_tensor("wv", [D, D], F32R, kind="ExternalInput").ap()
    wo_d = nc.dram_tensor("wo", [D, D], BF16, kind="ExternalInput").ap()
    bqk_d = nc.dram_tensor("bqk", [P, 2 * NCHUNK], F32, kind="ExternalInput").ap()
    bv_d = nc.dram_tensor("bv", [P, D], F32, kind="ExternalInput").ap()
    bo_d = nc.dram_tensor("bo", [P, D], F32, kind="ExternalInput").ap()
    msk_d = nc.dram_tensor("mask8", [P, msk_len], BF16, kind="ExternalInput").ap()
    out_d = nc.dram_tensor("out", [S, D], F32, kind="ExternalOutput").ap()

    with tile.TileContext(nc) as tc:
        with (
            tc.tile_pool(name="w", bufs=18) as wpool,
            tc.tile_pool(name="big", bufs=2) as bigpool,
            tc.tile_pool(name="qk", bufs=8) as qkpool,
            tc.tile_pool(name="v", bufs=8) as vpool,
            tc.tile_pool(name="cst", bufs=1) as cstpool,
            tc.tile_pool(name="exp", bufs=5) as exppool,
            tc.tile_pool(name="rcp", bufs=1) as rcppool,
            tc.tile_pool(name="osb", bufs=3) as osbpool,
            tc.tile_pool(name="pp", bufs=2, space="PSUM") as pp,
            tc.tile_pool(name="po", bufs=2, space="PSUM") as po,
        ):
            for _rep in range(repeats):
                # ---- warm-up on a memset tile: PE ramping from t~0 ----
                wup = cstpool.tile([P, P], BF16, tag="wup")
                nc.vector.memset(wup[:], 0.0)
                wps = pp.tile([P, S], F32, tag="pp", name="warmup_ps")
                for wi in range(34):
                    nc.tensor.matmul(
                        wps[:, 0:P], wup[:], wup[:], start=True, stop=True
                    )

                # ---- startup DMAs, ordered by first use ----
                xtq = [
                    bigpool.tile([P, 4, S], F32R, tag="big", name=f"xtq_{g}")
                    for g in range(2)
                ]

                def x_part(g, hf, c0, c1):
                    """One DMA for x chunks 4g+c0..4g+c1-1, column half hf."""
                    sl = slice(hf * HALF, (hf + 1) * HALF)
                    nc.sync.dma_start(
                        xtq[g][:, c0:c1, sl],
                        xt_d[
                            g * HALF + c0 * P : g * HALF + c1 * P, sl
                        ].rearrange("(c p) q -> p c q", p=P),
                    )

                def whalf(nm, w_dram, hf, lo=0, hi_=NCHUNK, dt=F32R):
                    """Half-strips [128, 512] of W columns [hf*512, (hf+1)*512)."""
                    ts = []
                    sl = slice(hf * HALF, (hf + 1) * HALF)
                    for dc in range(lo, hi_):
                        t = wpool.tile([P, HALF], dt, tag="w", name=f"{nm}{hf}_{dc}")
                        nc.sync.dma_start(t[:], w_dram[dc * P : (dc + 1) * P, sl])
                        ts.append(t)
                    return ts

                x_part(0, 0, 0, 1)
                qh0 = whalf("wq", wq_d, 0, 0, 1)
                x_part(0, 0, 1, 4)
                qh0 += whalf("wq", wq_d, 0, 1, 4)
                x_part(1, 0, 0, 4)
                qh0 += whalf("wq", wq_d, 0, 4, 8)
                bqk = cstpool.tile([P, 2 * NCHUNK], F32, tag="bqk")
                nc.sync.dma_start(bqk[:], bqk_d[:])
                x_part(0, 1, 0, 4)
                x_part(1, 1, 0, 4)
                kh0 = whalf("wk", wk_d, 0)
                bias = {}
                # bv (v-proj) and bo (o_proj) lifetimes don't overlap: share slot
                bias["bv"] = cstpool.tile([P, D], F32, tag="bvbo", name="bv_bc")
                nc.sync.dma_start(bias["bv"][:], bv_d[:])
                msk = cstpool.tile([P, msk_len], BF16, tag="msk")
                nc.sync.dma_start(msk[:], msk_d[:])
                xt = [xtq[dc // 4][:, dc % 4, :] for dc in range(NCHUNK)]

                # ---- helper: dense [d', s] projection (qT / kT), bf16 out ----
                def proj_half(whalf_tiles, chalf, bcol0, out_tag):
                    """qT/kT chunks chalf*4 .. chalf*4+3 from one W column half."""
                    outs = []
                    for cp in range(2):
                        cs = (chalf * 4 + 2 * cp, chalf * 4 + 2 * cp + 1)
                        pss = {
                            c: pp.tile([P, S], F32, tag="pp", name=f"ps_{out_tag}_{c}")
                            for c in cs
                        }
                        for j in range(2):
                            sl = slice(j * HALF, (j + 1) * HALF)
                            for c in cs:
                                lc = (c % 4) * P
                                for dc in range(NCHUNK):
                                    nc.tensor.matmul(
                                        pss[c][:, sl],
                                        whalf_tiles[dc][:, lc : lc + P],
                                        xt[dc][:, sl],
                                        start=(dc == 0),
                                        stop=(dc == NCHUNK - 1),
                                    )
                        for c in cs:
                            o = qkpool.tile(
                                [P, S], F32R, tag=out_tag, name=f"{out_tag}_{c}"
                            )
                            nc.vector.tensor_add(
                                o[:],
                                pss[c][:],
                                bqk[:, bcol0 + c : bcol0 + c + 1].to_broadcast((P, S)),
                            )
                            outs.append(o)
                    return outs

                with nc.named_scope("qk_proj"):
                    qT = proj_half(qh0, 0, 0, "qT")
                    qh1 = whalf("wq", wq_d, 1)
                    kT = proj_half(kh0, 0, NCHUNK, "kT")
                    kh1 = whalf("wk", wk_d, 1)
                    qT += proj_half(qh1, 1, 0, "qT")
                    kT += proj_half(kh1, 1, NCHUNK, "kT")

                # ---- v projection: [s, 16, 128] bf16, cols 64:128 = 1.0 ----
                with nc.named_scope("v_proj"):
                    vh = [whalf("wv", wv_d, 0), whalf("wv", wv_d, 1)]
                    vtiles = []
                    for sc in range(NCHUNK):
                        vt = vpool.tile([P, H, 2 * DK], BF16, tag="v")
                        nc.vector.memset(vt[:, :, DK : 2 * DK], 1.0)
                        ps = pp.tile([P, S], F32, tag="pp")
                        for j in range(2):
                            sl = slice(j * HALF, (j + 1) * HALF)
                            for dc in range(NCHUNK):
                                nc.tensor.matmul(
                                    ps[:, sl],
                                    xt[dc][:, sc * P : (sc + 1) * P],
                                    vh[j][dc][:],
                                    start=(dc == 0),
                                    stop=(dc == NCHUNK - 1),
                                )
                        nc.vector.tensor_add(
                            vt[:, :, 0:DK],
                            ps[:].rearrange("p (h d) -> p h d", h=H),
                            bias["bv"][:].rearrange("p (h d) -> p h d", h=H),
                        )
                        vtiles.append(vt)

                # ---- attention heads ----
                bias["bo"] = cstpool.tile([P, D], F32, tag="bvbo", name="bo_bc")
                nc.sync.dma_start(bias["bo"][:], bo_d[:])
                attn = [None, None]

                # Wo bf16 strips prefetched before the head loop.
                oh = [whalf("wo", wo_d, 0, dt=BF16), whalf("wo", wo_d, 1, dt=BF16)]

                def emit_scores_exp(h, kc):
                    """scores on PE, exp on ACT (bf16 out), 0/1 bf16 masks on DVE."""
                    c, r = h // 2, (h % 2) * DK
                    pss = pp.tile([P, S], F32, tag="pp", name=f"pss_{h}_{kc}")
                    lhs = kT[c][r : r + DK, kc * P : (kc + 1) * P]
                    for lo in range(0, hi_s[kc], HALF):
                        sl = slice(lo, min(lo + HALF, hi_s[kc]))
                        nc.tensor.matmul(
                            pss[:, sl],
                            lhs,
                            qT[c][r : r + DK, sl],
                            start=True,
                            stop=True,
                        )
                    et = exppool.tile([P, S], BF16, tag="exp", name=f"et_{h}_{kc}")
                    nc.scalar.activation(
                        et[:, 0 : hi_e[kc]], pss[:, 0 : hi_e[kc]], EXP
                    )
                    # one 0/1 mask mult over cols [kc*128, hi): diag pattern on
                    # the diagonal block, column mask below the diagonal
                    w = hi[kc] - kc * P
                    off = msk_off[kc]
                    nc.vector.tensor_mul(
                        et[:, kc * P : hi[kc]],
                        et[:, kc * P : hi[kc]],
                        msk[:, off : off + w],
                    )
                    return et

                def emit_pv(h, kc, pso, et):
                    # kc descending: ranges nest. PSUM start=True zeroing is
                    # BANK-granular, so the first writer (kc=7, whose range is
                    # always full) must cover each 512-col bank in ONE
                    # start=True matmul; later kc's accumulate prefixes of the
                    # bank with start=False. stop=True goes on each bank's
                    # last writer (the smallest kc that still reaches it).
                    first = kc == NCHUNK - 1
                    for b0 in range(0, hi_p[kc], HALF):
                        hi2 = min(b0 + HALF, S) if first else min(b0 + HALF, hi_p[kc])
                        last = all(hi_p[k2] <= b0 for k2 in range(kc))
                        nc.tensor.matmul(
                            pso[:, b0:hi2],
                            vtiles[kc][:, h, :],
                            et[:, b0:hi2],
                            start=first,
                            stop=last,
                        )

                normq = deque()

                def emit_norm(h, pso):
                    """Reciprocal now; queue the 4 scramble muls for staggered
                    emission (drained one per pop) so the DVE isn't bursty at
                    head boundaries, starving the mask-mults that gate PV."""
                    rcp = rcppool.tile([DK, S], F32, tag="rcp", name=f"rcp_{h}")
                    nc.vector.reciprocal(rcp[:], pso[DK : 2 * DK, :])
                    # attn[g][e*64+d, cc, h*64+u] = O_h[u*16 + 2*(4g+cc) + e, d]/denom
                    src = pso[0:DK, :].rearrange("d (u j) -> d j u", j=16)
                    rbs = rcp[:].rearrange("d (u j) -> d j u", j=16)
                    for g in range(2):
                        if attn[g] is None:
                            attn[g] = bigpool.tile(
                                [P, 4, S], BF16, tag="big", name=f"attnq_{g}"
                            )
                        for e in range(2):
                            jsl = slice(8 * g + e, 8 * (g + 1), 2)

                            def mul(g=g, e=e, jsl=jsl, src=src, rbs=rbs, h=h):
                                nc.vector.tensor_mul(
                                    attn[g][
                                        e * DK : (e + 1) * DK, :, h * DK : (h + 1) * DK
                                    ],
                                    src[:, jsl, :],
                                    rbs[:, jsl, :],
                                )

                            normq.append((h, mul))

                def drain_one_norm():
                    if normq:
                        normq.popleft()[1]()

                def flush_norm(upto_h):
                    while normq and normq[0][0] <= upto_h:
                        normq.popleft()[1]()

                oq = deque()

                def emit_oproj(sc, spread=False):
                    """o_proj chunk sc. spread=True queues per-cc pieces that
                    are drained ahead of each tile's scores matmuls, so the PE
                    has work in program order while it waits on the scores
                    psum ring (exp + semaphore round-trip)."""
                    ps = po.tile([P, S], F32, tag="po", name=f"psf_{sc}")
                    for j in range(2):
                        sl = slice(j * HALF, (j + 1) * HALF)
                        for cc in range(NCHUNK):

                            def mm(j=j, cc=cc, sl=sl):
                                nc.tensor.matmul(
                                    ps[:, sl],
                                    attn[cc // 4][:, cc % 4, sc * P : (sc + 1) * P],
                                    oh[j][cc][:],
                                    start=(cc == 0),
                                    stop=(cc == NCHUNK - 1),
                                )

                            oq.append(mm) if spread else mm()

                        def biasdma(j=j, sl=sl):
                            ot = osbpool.tile(
                                [P, HALF], F32, tag="osb", name=f"ot_{sc}_{j}"
                            )
                            nc.vector.tensor_add(ot[:], ps[:, sl], bias["bo"][:, sl])
                            nc.sync.dma_start(out_d[sc * P : (sc + 1) * P, sl], ot[:])

                        oq.append(biasdma) if spread else biasdma()

                def drain_oq(n=2):
                    for _ in range(n):
                        if oq:
                            oq.popleft()()

                # Flat (h, kc-descending) stream, PV lagging scores/exp so the
                # in-order PE never waits on a just-issued exp. o_proj chunk k
                # (needs heads 2k,2k+1 only) runs two heads later; chunk 6 is
                # held for the final norm window so the PE stays busy at the
                # tail (holding more would recycle pso_15's psum slot and
                # serialize on the last norm).
                pend = deque()
                pso_cur = None

                def pop_pv():
                    ph, pkc, ppso, pet = pend.popleft()
                    emit_pv(ph, pkc, ppso, pet)
                    drain_one_norm()
                    if pkc == 0:
                        emit_norm(ph, ppso)
                        if ph % 2 == 1 and 3 <= ph <= 13:
                            sc = (ph - 3) // 2
                            flush_norm(2 * sc + 1)
                            emit_oproj(sc, spread=True)

                for h in range(H):
                    pso_cur = po.tile([P, S], F32, tag="po", name=f"pso_{h}")
                    for kc in range(NCHUNK - 1, -1, -1):
                        drain_oq()
                        et = emit_scores_exp(h, kc)
                        if len(pend) >= 4:
                            pop_pv()
                        pend.append((h, kc, pso_cur, et))
                while len(pend) > 1:
                    pop_pv()
                # last PV of head 15: norm emitted first so DVE starts at once;
                # o_proj chunk 6 keeps the PE busy under the final norm chain.
                ph, pkc, ppso, pet = pend.popleft()
                emit_pv(ph, pkc, ppso, pet)
                emit_norm(ph, ppso)
                drain_oq(len(oq))
                flush_norm(H - 2)
                emit_oproj(NCHUNK - 2)
                flush_norm(H)
                emit_oproj(NCHUNK - 1)

    nc.compile()
    return nc


def _host_masks(prefix_b: int, trim: int):
    """Combined multiplicative 0/1 mask, bf16, applied to exp output.

    For scores-T tile kc (cols q in [kc*128, hi_kc)): element (i, q) keeps
    exp iff allowed(q, k=kc*128+i) = (q < prefix) or (k >= q).
    """
    hi = _tile_hi(trim)
    i = np.arange(P)[:, None]
    segs = []
    for kc in range(NCHUNK):
        q = np.arange(kc * P, hi[kc])[None, :]
        k = kc * P + i
        allowed = (q < prefix_b) | (k >= q)
        segs.append(allowed.astype(ml_dtypes.bfloat16))
    return np.concatenate(segs, axis=1)


def kernel(x, prefix, Wq, bq, Wk, bk, Wv, bv, Wo, bo, _trace=False):
    x = np.asarray(x, dtype=np.float32)
    prefix = np.asarray(prefix)
    Wq, Wk, Wv = (
        np.ascontiguousarray(np.asarray(w, np.float32)) for w in (Wq, Wk, Wv)
    )
    Wo16 = np.ascontiguousarray(
        np.asarray(Wo, np.float32).astype(ml_dtypes.bfloat16)
    )
    bv, bo = (
        np.broadcast_to(np.asarray(v, np.float32).reshape(1, D), (P, D)).copy()
        for v in (bv, bo)
    )
    bqk = np.stack(
        [np.asarray(bq, np.float32).reshape(NCHUNK, P), np.asarray(bk, np.float32).reshape(NCHUNK, P)], axis=0
    ).reshape(2 * NCHUNK, P).T.copy()  # [128, 16]: cols 0-7 = bq chunks, 8-15 = bk

    # cols q >= max(prefix) below the diagonal are masked on every core:
    # specialize (and cache) the program on that bound.
    trim = int(prefix.max())
    if _CACHED.get("trim") != trim:
        _CACHED["nc"] = build_nc(trim=trim)
        _CACHED["trim"] = trim
    nc = _CACHED["nc"]

    in_maps = []
    for b in range(B):
        mask8 = _host_masks(int(prefix[b]), trim)
        in_maps.append(
            {
                "xt": np.ascontiguousarray(x[b].T),
                "wq": Wq, "wk": Wk, "wv": Wv, "wo": Wo16,
                "bqk": bqk, "bv": bv, "bo": bo,
                "mask8": mask8,
            }
        )

    res = run_bass_kernel_spmd(nc, in_maps, core_ids=list(range(NCORES)), trace=_trace)
    out = np.stack([res.results[b]["out"] for b in range(B)], axis=0)
    if _trace:
        return out, res
    return out
